# revision 17
# baseline (speedup 1.0000x reference)
"""Trainium2 Bass kernel for nn_Attention_18786186952997.

Dense causal-attention transformer block with ternarized (BitNet-style)
weights and RoPE:

    wq = ternarize(w_qkv); wp = ternarize(w_proj)
    qkv = x @ wq.T ; q,k,v split ; RoPE(q,k) ; causal SDPA ; y @ wp.T

Sharding: 8 cores = 2 batches x 4 head-groups (4 heads each).  Each core
computes its batch's qkv projections for its 4 heads, runs causal
flash-style attention fully on-chip, and produces a partial (transposed)
projection output; the host sums the 4 partials per batch.

Device compute layout is channel-major ("transposed"): q.T/k.T are
produced as [head_dim, tokens] so RoPE and QK^T need no on-chip
transposes; exp(scores.T) is exactly the stationary layout that A@V
needs; softmax denominators come free from ones-columns appended to V.
Ternary weights are passed as exact bf16 sign matrices; the abs-mean
scales are folded into the exp() scale and the final output scale.
"""

import os
import sys
import types

import numpy as np

sys.path.insert(0, "/opt/trn_rl_repo")

import ml_dtypes  # noqa: E402

BF16 = ml_dtypes.bfloat16

B, T, C, H, D = 2, 2048, 1024, 16, 64
N_CORES = 8
HEADS_PER_CORE = 4
P = 128
QT = 512            # q tile (moving free dim)
NQT = T // QT       # 4
NKC = T // P        # 16 k chunks
NCC = C // P        # 8 contraction chunks

_CACHE = {}


def _install_ntff_hook():
    """bass_utils' trace=True path needs antenv.axon_hooks, absent in this
    image; synthesize it around the boot module's ctypes hook."""
    if "antenv.axon_hooks" in sys.modules:
        return
    try:
        import antenv  # noqa: F401
        from trn_agent_boot.trn_boot import _ntff_profile_via_ctypes
    except Exception:
        return
    mod = types.ModuleType("antenv.axon_hooks")
    holder = {}
    mod.set_axon_ntff_profile_hook = lambda h: holder.__setitem__("h", h)
    mod.get_axon_ntff_profile_hook = lambda: holder.get("h")
    sys.modules["antenv.axon_hooks"] = mod
    sys.modules["antenv"].axon_hooks = mod
    try:
        hook = _ntff_profile_via_ctypes("/opt/axon/libaxon_pjrt.so")
        mod.set_axon_ntff_profile_hook(hook)
    except Exception:
        pass


def _ternarize_host(w):
    """Sign matrix and abs-mean scale, bit-matching the jax reference."""
    try:
        import jax.numpy as jnp

        wj = jnp.asarray(w)
        am = jnp.maximum(jnp.abs(wj).mean(), 1e-5)
        thr = 0.7 * am
        s = jnp.where(wj > thr, 1.0, jnp.where(wj < -thr, -1.0, 0.0))
        return np.asarray(s, dtype=np.float32), np.float32(am)
    except Exception:
        am = np.float32(max(np.abs(w).astype(np.float32).mean(dtype=np.float32), 1e-5))
        thr = np.float32(0.7) * am
        s = np.where(w > thr, 1.0, np.where(w < -thr, -1.0, 0.0)).astype(np.float32)
        return s, am


def _build_program():
    import concourse.bass as bass  # noqa: F401
    import concourse.mybir as mybir
    import concourse.tile as tile
    from concourse import bacc

    F32 = mybir.dt.float32
    BF = mybir.dt.bfloat16
    AF = mybir.ActivationFunctionType

    nc = bacc.Bacc("TRN2", target_bir_lowering=False, debug=False,
                   num_devices=N_CORES)

    xt = nc.dram_tensor("xt", [C, T], BF, kind="ExternalInput").ap()
    wqk = nc.dram_tensor("wqk", [C, 512], BF, kind="ExternalInput").ap()
    wv = nc.dram_tensor("wv", [C, 256], BF, kind="ExternalInput").ap()
    wp = nc.dram_tensor("wp", [256, 1024], BF, kind="ExternalInput").ap()
    cos2 = nc.dram_tensor("cos2", [P, T], F32, kind="ExternalInput").ap()
    ss2 = nc.dram_tensor("ss2", [P, T], F32, kind="ExternalInput").ap()
    sc_exp = nc.dram_tensor("sc_exp", [P, 1], F32, kind="ExternalInput").ap()
    sc_out = nc.dram_tensor("sc_out", [P, 1], F32, kind="ExternalInput").ap()
    outT = nc.dram_tensor("outT", [C, T], F32, kind="ExternalOutput").ap()

    with tile.TileContext(nc) as tc:
        with (
            tc.tile_pool(name="consts", bufs=1) as consts,
            tc.tile_pool(name="tmps", bufs=10) as tmps,
            tc.tile_pool(name="epool", bufs=8) as epool,
            tc.tile_pool(name="opool", bufs=6) as opool,
            tc.tile_pool(name="ps_sc", bufs=2, space="PSUM") as ps_sc,
            tc.tile_pool(name="ps_y", bufs=4, space="PSUM") as ps_y,
        ):
            # ---- persistent SBUF loads, interleaved so accumulation
            # chains can start as soon as their chunk lands ----
            sce_sb = consts.tile([P, 1], F32)
            nc.sync.dma_start(out=sce_sb, in_=sc_exp[:])
            sco_sb = consts.tile([P, 1], F32)
            nc.sync.dma_start(out=sco_sb, in_=sc_out[:])
            cos_sb = consts.tile([P, T], F32)
            ss_sb = consts.tile([P, T], F32)
            nc.scalar.dma_start(out=cos_sb[0:64, :], in_=cos2[0:64, :])
            nc.scalar.dma_start(out=cos_sb[64:128, :], in_=cos2[0:64, :])
            nc.scalar.dma_start(out=ss_sb[0:64, :], in_=ss2[0:64, :])
            nc.scalar.dma_start(out=ss_sb[64:128, :], in_=ss2[0:64, :])

            xt_sb = consts.tile([P, NCC, T], BF)
            xt_p = xt.rearrange("(n p) t -> p n t", p=P)
            wqk_sb = consts.tile([P, NCC, 512], BF)
            wqk_p = wqk.rearrange("(n p) m -> p n m", p=P)
            wv_sb = consts.tile([P, NCC, 256], BF)
            wv_p = wv.rearrange("(n p) m -> p n m", p=P)
            # three queues: sync + gpsimd carry x, scalar carries weights
            for i in range(NCC):
                eng = nc.sync if i % 2 == 0 else nc.gpsimd
                eng.dma_start(out=xt_sb[:, i, :], in_=xt_p[:, i, :])
                nc.scalar.dma_start(out=wqk_sb[:, i, :], in_=wqk_p[:, i, :])
                nc.scalar.dma_start(out=wv_sb[:, i, :], in_=wv_p[:, i, :])
            wp_sb = consts.tile([P, 2, 1024], BF)
            wp_p = wp.rearrange("(n p) m -> p n m", p=P)
            nc.sync.dma_start(out=wp_sb, in_=wp_p)

            qk_sb = consts.tile([P, 4, T], BF)  # blk: q01, q23, k01, k23
            v_sb = consts.tile([P, NKC, 2, 256], BF)
            y_sb = consts.tile([P, 2, T], BF)
            # per head: [ones(64) | v(64)] -> denominators at psum rows 0:64
            v_sb4 = v_sb.rearrange("p n g (h o d) -> p n g h o d", h=2, o=2)
            nc.gpsimd.memset(v_sb4[:, :, :, :, 0, :], 1.0)

            def emit_qkv(qt):
                # wqk col blocks: QA[0:256) KA[256:512)
                qs = slice(qt * QT, (qt + 1) * QT)
                for pair in range(2):
                    blks = ((0, 2), (1, 3))[pair]
                    ps = ps_sc.tile([P, 1024], F32, tag="sc", name="qkvps")
                    for kc in range(NCC):
                        for j, blk in enumerate(blks):
                            base_a = [0, 128, 256, 384][blk]
                            nc.tensor.matmul(
                                ps[:, j * QT:(j + 1) * QT],
                                lhsT=wqk_sb[:, kc, base_a:base_a + P],
                                rhs=xt_sb[:, kc, qs],
                                start=(kc == 0),
                                stop=(kc == NCC - 1),
                            )
                    for j, blk in enumerate(blks):
                        pj = ps[:, j * QT:(j + 1) * QT]
                        t1 = tmps.tile([P, QT], F32, tag="t1")
                        nc.vector.tensor_mul(t1, pj, cos_sb[:, qs])
                        # t2[d] = q[perm(d)] * ss[d]: 4 shifted psum muls
                        t2 = tmps.tile([P, QT], F32, tag="t2")
                        for hb in range(2):
                            o0 = hb * 64
                            nc.vector.tensor_mul(
                                t2[o0:o0 + 32, :], pj[o0 + 32:o0 + 64, :],
                                ss_sb[o0:o0 + 32, qs])
                            nc.vector.tensor_mul(
                                t2[o0 + 32:o0 + 64, :], pj[o0:o0 + 32, :],
                                ss_sb[o0 + 32:o0 + 64, qs])
                        nc.gpsimd.tensor_add(qk_sb[:, blk, qs], t1, t2)

            def emit_v(tt):
                vp = ps_y.tile([P, 256], F32, tag="y")
                for kc in range(NCC):
                    nc.tensor.matmul(
                        vp,
                        lhsT=xt_sb[:, kc, tt * P:(tt + 1) * P],
                        rhs=wv_sb[:, kc, :],
                        start=(kc == 0),
                        stop=(kc == NCC - 1),
                    )
                vp4 = vp.rearrange("p (g h d) -> p g h d", g=2, h=2)
                nc.vector.tensor_copy(v_sb4[:, tt, :, :, 1, :], vp4)

            def emit_attn(grp, qt):
                q_t = qk_sb[:, grp, :]
                k_t = qk_sb[:, 2 + grp, :]
                qs = slice(qt * QT, (qt + 1) * QT)
                KC = 4 * (qt + 1)  # causal k chunks
                yA = ps_y.tile([P, QT], F32, tag="y")
                yB = ps_y.tile([P, QT], F32, tag="y")
                for kc in range(KC):
                    ks = slice(kc * P, (kc + 1) * P)
                    ps = ps_sc.tile([P, 1024], F32, tag="sc")
                    nc.tensor.matmul(ps[:, 0:QT], lhsT=k_t[0:64, ks],
                                     rhs=q_t[0:64, qs],
                                     start=True, stop=True)
                    nc.tensor.matmul(ps[:, QT:1024], lhsT=k_t[64:128, ks],
                                     rhs=q_t[64:128, qs],
                                     start=True, stop=True)
                    e = epool.tile([P, 1024], BF, tag="e")
                    delta = kc * P - qt * QT
                    if delta <= 0:
                        nc.scalar.activation(e, ps, AF.Exp,
                                             scale=sce_sb[:, 0:1])
                    else:
                        # diagonal tile: columns below delta are fully
                        # masked; exp only the live range, zero the rest
                        e2 = e.rearrange("p (j f) -> p j f", j=2)
                        p2 = ps.rearrange("p (j f) -> p j f", j=2)
                        nc.gpsimd.memset(e2[:, :, 0:delta], 0.0)
                        nc.scalar.activation(e2[:, :, delta:QT],
                                             p2[:, :, delta:QT],
                                             AF.Exp, scale=sce_sb[:, 0:1])
                    if delta >= 0:
                        # keep where f - p - delta >= 0
                        e2 = e.rearrange("p (j f) -> p j f", j=2)
                        nc.gpsimd.affine_select(
                            e2, e2,
                            pattern=[[0, 2], [1, QT]],
                            compare_op=mybir.AluOpType.is_ge,
                            fill=0.0,
                            base=-delta,
                            channel_multiplier=-1,
                        )
                    nc.tensor.matmul(yA, lhsT=v_sb[:, kc, grp, 0:128],
                                     rhs=e[:, 0:QT],
                                     start=(kc == 0), stop=(kc == KC - 1))
                    nc.tensor.matmul(yB, lhsT=v_sb[:, kc, grp, 128:256],
                                     rhs=e[:, QT:1024],
                                     start=(kc == 0), stop=(kc == KC - 1))
                # both heads: denom rows 0:64, y rows 64:128
                rcA = tmps.tile([P, QT], F32, tag="rc")
                nc.vector.reciprocal_approx_fast(rcA[0:64, :], yA[0:64, :])
                nc.vector.tensor_mul(y_sb[0:64, grp, qs], yA[64:128, :],
                                     rcA[0:64, :])
                rcB = tmps.tile([P, QT], F32, tag="rc")
                nc.vector.reciprocal_approx_fast(rcB[0:64, :], yB[0:64, :])
                nc.vector.tensor_copy(rcB[64:128, :], rcB[0:64, :])
                nc.vector.tensor_mul(y_sb[64:128, grp, qs], yB[64:128, :],
                                     rcB[64:128, :])

            def emit_proj(qt):
                qs = slice(qt * QT, (qt + 1) * QT)
                for mt in range(8):
                    ms = slice(mt * P, (mt + 1) * P)
                    pp = ps_y.tile([P, QT], F32, tag="y")
                    for ch in range(2):
                        nc.tensor.matmul(pp, lhsT=wp_sb[:, ch, ms],
                                         rhs=y_sb[:, ch, qs],
                                         start=(ch == 0), stop=(ch == 1))
                    ot = opool.tile([P, QT], F32, tag="ot")
                    if mt % 2 == 0:
                        nc.scalar.activation(ot, pp, AF.Copy,
                                             scale=sco_sb[:, 0:1])
                    else:
                        nc.vector.tensor_scalar_mul(ot, pp, sco_sb[:, 0:1])
                    eng = (nc.sync, nc.gpsimd)[mt % 2]
                    eng.dma_start(out=outT[ms, qs], in_=ot)

            # qt order (1,2,3,0): prefix deps stay legal while the kernel
            # tail is the lightest attention tile + its projection
            emit_qkv(0)
            emit_qkv(1)
            for tt in range(8):
                emit_v(tt)
            emit_attn(0, 1)
            emit_qkv(2)
            emit_attn(1, 1)
            for tt in range(8, 12):
                emit_v(tt)
            emit_proj(1)
            emit_attn(0, 2)
            emit_qkv(3)
            emit_attn(1, 2)
            for tt in range(12, 16):
                emit_v(tt)
            emit_proj(2)
            emit_attn(0, 3)
            emit_attn(1, 3)
            emit_proj(3)
            emit_attn(0, 0)
            emit_attn(1, 0)
            emit_proj(0)

    nc.finalize()
    return nc


def _prep_inputs(x, cos, sin, w_qkv, w_proj):
    sq, am_q = _ternarize_host(w_qkv)
    sp, am_p = _ternarize_host(w_proj)

    cos_t = np.ascontiguousarray(cos[0, 0].T).astype(np.float32)  # [D, T]
    sin_t = np.ascontiguousarray(sin[0, 0].T).astype(np.float32)
    sgn = np.where(np.arange(D)[:, None] < D // 2, np.float32(-1.0),
                   np.float32(1.0))
    ss_t = sin_t * sgn
    cos2 = np.concatenate([cos_t, cos_t], axis=0)          # [128, T]
    ss2 = np.concatenate([ss_t, ss_t], axis=0)
    sc_exp = np.full((P, 1), am_q * am_q / np.sqrt(np.float32(D)),
                     np.float32)
    sc_out = np.full((P, 1), am_q * am_p, np.float32)

    perm = (np.arange(D) + D // 2) % D
    in_maps = []
    for core in range(N_CORES):
        b, g = divmod(core, HEADS_PER_CORE)
        heads = [4 * g + h for h in range(4)]
        q_rows = np.concatenate([np.arange(h * D, (h + 1) * D) for h in heads])
        k_rows = C + q_rows
        v_rows = 2 * C + q_rows
        wqk_block = np.concatenate([sq[q_rows], sq[k_rows]], axis=0)
        wqk_t = np.ascontiguousarray(wqk_block.T).astype(BF16)   # [C, 1024]
        wv_t = np.ascontiguousarray(sq[v_rows].T).astype(BF16)   # [C, 256]
        wp_t = np.ascontiguousarray(sp[:, q_rows].T).astype(BF16)  # [256, C]
        xt = np.ascontiguousarray(x[b].T).astype(BF16)           # [C, T]
        in_maps.append({
            "xt": xt, "wqk": wqk_t, "wv": wv_t, "wp": wp_t,
            "cos2": cos2, "ss2": ss2, "sc_exp": sc_exp, "sc_out": sc_out,
        })
    return in_maps


def kernel(x, cos, sin, w_qkv, w_proj):
    x = np.asarray(x, dtype=np.float32)
    cos = np.asarray(cos, dtype=np.float32)
    sin = np.asarray(sin, dtype=np.float32)
    w_qkv = np.asarray(w_qkv, dtype=np.float32)
    w_proj = np.asarray(w_proj, dtype=np.float32)

    _install_ntff_hook()
    from concourse.bass_utils import run_bass_kernel_spmd

    if "nc" not in _CACHE:
        _CACHE["nc"] = _build_program()
    nc = _CACHE["nc"]

    in_maps = _prep_inputs(x, cos, sin, w_qkv, w_proj)
    trace = bool(os.environ.get("KERNEL_TRACE"))
    res = run_bass_kernel_spmd(nc, in_maps, core_ids=list(range(N_CORES)),
                               trace=trace)
    _CACHE["exec_time_ns"] = res.exec_time_ns

    out = np.zeros((B, T, C), dtype=np.float32)
    for core in range(N_CORES):
        b = core // HEADS_PER_CORE
        out[b] += res.results[core]["outT"].T
    return out


# revision 18
# speedup vs baseline: 1.0144x; 1.0144x over previous
"""Trainium2 Bass kernel for nn_Attention_18786186952997.

Dense causal-attention transformer block with ternarized (BitNet-style)
weights and RoPE:

    wq = ternarize(w_qkv); wp = ternarize(w_proj)
    qkv = x @ wq.T ; q,k,v split ; RoPE(q,k) ; causal SDPA ; y @ wp.T

Sharding: 8 cores = 2 batches x 4 head-groups (4 heads each).  Each core
computes its batch's qkv projections for its 4 heads, runs causal
flash-style attention fully on-chip, and produces a partial (transposed)
projection output; the host sums the 4 partials per batch.

Device compute layout is channel-major ("transposed"): q.T/k.T are
produced as [head_dim, tokens] so RoPE and QK^T need no on-chip
transposes; exp(scores.T) is exactly the stationary layout that A@V
needs; softmax denominators come free from ones-columns appended to V.
Ternary weights are passed as exact bf16 sign matrices; the abs-mean
scales are folded into the exp() scale and the final output scale.
"""

import os
import sys
import types

import numpy as np

sys.path.insert(0, "/opt/trn_rl_repo")

import ml_dtypes  # noqa: E402

BF16 = ml_dtypes.bfloat16

B, T, C, H, D = 2, 2048, 1024, 16, 64
N_CORES = 8
HEADS_PER_CORE = 4
P = 128
QT = 512            # q tile (moving free dim)
NQT = T // QT       # 4
NKC = T // P        # 16 k chunks
NCC = C // P        # 8 contraction chunks

_CACHE = {}


def _install_ntff_hook():
    """bass_utils' trace=True path needs antenv.axon_hooks, absent in this
    image; synthesize it around the boot module's ctypes hook."""
    if "antenv.axon_hooks" in sys.modules:
        return
    try:
        import antenv  # noqa: F401
        from trn_agent_boot.trn_boot import _ntff_profile_via_ctypes
    except Exception:
        return
    mod = types.ModuleType("antenv.axon_hooks")
    holder = {}
    mod.set_axon_ntff_profile_hook = lambda h: holder.__setitem__("h", h)
    mod.get_axon_ntff_profile_hook = lambda: holder.get("h")
    sys.modules["antenv.axon_hooks"] = mod
    sys.modules["antenv"].axon_hooks = mod
    try:
        hook = _ntff_profile_via_ctypes("/opt/axon/libaxon_pjrt.so")
        mod.set_axon_ntff_profile_hook(hook)
    except Exception:
        pass


def _ternarize_host(w):
    """Sign matrix and abs-mean scale, bit-matching the jax reference."""
    try:
        import jax.numpy as jnp

        wj = jnp.asarray(w)
        am = jnp.maximum(jnp.abs(wj).mean(), 1e-5)
        thr = 0.7 * am
        s = jnp.where(wj > thr, 1.0, jnp.where(wj < -thr, -1.0, 0.0))
        return np.asarray(s, dtype=np.float32), np.float32(am)
    except Exception:
        am = np.float32(max(np.abs(w).astype(np.float32).mean(dtype=np.float32), 1e-5))
        thr = np.float32(0.7) * am
        s = np.where(w > thr, 1.0, np.where(w < -thr, -1.0, 0.0)).astype(np.float32)
        return s, am


def _build_program():
    import concourse.bass as bass  # noqa: F401
    import concourse.mybir as mybir
    import concourse.tile as tile
    from concourse import bacc

    F32 = mybir.dt.float32
    BF = mybir.dt.bfloat16
    AF = mybir.ActivationFunctionType

    nc = bacc.Bacc("TRN2", target_bir_lowering=False, debug=False,
                   num_devices=N_CORES)

    xt = nc.dram_tensor("xt", [C, T], BF, kind="ExternalInput").ap()
    wqk = nc.dram_tensor("wqk", [C, 512], BF, kind="ExternalInput").ap()
    wv = nc.dram_tensor("wv", [C, 256], BF, kind="ExternalInput").ap()
    wp = nc.dram_tensor("wp", [256, 1024], BF, kind="ExternalInput").ap()
    cos2 = nc.dram_tensor("cos2", [P, T], F32, kind="ExternalInput").ap()
    ss2 = nc.dram_tensor("ss2", [P, T], F32, kind="ExternalInput").ap()
    sc_exp = nc.dram_tensor("sc_exp", [P, 1], F32, kind="ExternalInput").ap()
    sc_out = nc.dram_tensor("sc_out", [P, 1], F32, kind="ExternalInput").ap()
    outT = nc.dram_tensor("outT", [C, T], F32, kind="ExternalOutput").ap()

    with tile.TileContext(nc) as tc:
        with (
            tc.tile_pool(name="consts", bufs=1) as consts,
            tc.tile_pool(name="tmps", bufs=10) as tmps,
            tc.tile_pool(name="epool", bufs=8) as epool,
            tc.tile_pool(name="opool", bufs=6) as opool,
            tc.tile_pool(name="ps_sc", bufs=3, space="PSUM") as ps_sc,
            tc.tile_pool(name="ps_y", bufs=2, space="PSUM") as ps_y,
        ):
            # ---- persistent SBUF loads, interleaved so accumulation
            # chains can start as soon as their chunk lands ----
            sce_sb = consts.tile([P, 1], F32)
            nc.sync.dma_start(out=sce_sb, in_=sc_exp[:])
            sco_sb = consts.tile([P, 1], F32)
            nc.sync.dma_start(out=sco_sb, in_=sc_out[:])
            cos_sb = consts.tile([P, T], F32)
            ss_sb = consts.tile([P, T], F32)
            nc.scalar.dma_start(out=cos_sb[0:64, :], in_=cos2[0:64, :])
            nc.scalar.dma_start(out=cos_sb[64:128, :], in_=cos2[0:64, :])
            nc.scalar.dma_start(out=ss_sb[0:64, :], in_=ss2[0:64, :])
            nc.scalar.dma_start(out=ss_sb[64:128, :], in_=ss2[0:64, :])

            xt_sb = consts.tile([P, NCC, T], BF)
            xt_p = xt.rearrange("(n p) t -> p n t", p=P)
            wqk_sb = consts.tile([P, NCC, 512], BF)
            wqk_p = wqk.rearrange("(n p) m -> p n m", p=P)
            wv_sb = consts.tile([P, NCC, 256], BF)
            wv_p = wv.rearrange("(n p) m -> p n m", p=P)
            # three queues: sync + gpsimd carry x, scalar carries weights
            for i in range(NCC):
                eng = nc.sync if i % 2 == 0 else nc.gpsimd
                eng.dma_start(out=xt_sb[:, i, :], in_=xt_p[:, i, :])
                nc.scalar.dma_start(out=wqk_sb[:, i, :], in_=wqk_p[:, i, :])
                nc.scalar.dma_start(out=wv_sb[:, i, :], in_=wv_p[:, i, :])
            wp_sb = consts.tile([P, 2, 1024], BF)
            wp_p = wp.rearrange("(n p) m -> p n m", p=P)
            nc.sync.dma_start(out=wp_sb, in_=wp_p)

            qk_sb = consts.tile([P, 4, T], BF)  # blk: q01, q23, k01, k23
            v_sb = consts.tile([P, NKC, 2, 256], BF)
            y_sb = consts.tile([P, 2, T], BF)
            # per head: [ones(64) | v(64)] -> denominators at psum rows 0:64
            v_sb4 = v_sb.rearrange("p n g (h o d) -> p n g h o d", h=2, o=2)
            nc.gpsimd.memset(v_sb4[:, :, :, :, 0, :], 1.0)

            def emit_qkv(qt):
                # wqk col blocks: QA[0:256) KA[256:512)
                qs = slice(qt * QT, (qt + 1) * QT)
                for pair in range(2):
                    blks = ((0, 2), (1, 3))[pair]
                    ps = ps_sc.tile([P, 1024], F32, tag="sc", name="qkvps")
                    for kc in range(NCC):
                        for j, blk in enumerate(blks):
                            base_a = [0, 128, 256, 384][blk]
                            nc.tensor.matmul(
                                ps[:, j * QT:(j + 1) * QT],
                                lhsT=wqk_sb[:, kc, base_a:base_a + P],
                                rhs=xt_sb[:, kc, qs],
                                start=(kc == 0),
                                stop=(kc == NCC - 1),
                            )
                    for j, blk in enumerate(blks):
                        pj = ps[:, j * QT:(j + 1) * QT]
                        t1 = tmps.tile([P, QT], F32, tag="t1")
                        nc.vector.tensor_mul(t1, pj, cos_sb[:, qs])
                        # t2[d] = q[perm(d)] * ss[d]: 4 shifted psum muls
                        t2 = tmps.tile([P, QT], F32, tag="t2")
                        for hb in range(2):
                            o0 = hb * 64
                            nc.vector.tensor_mul(
                                t2[o0:o0 + 32, :], pj[o0 + 32:o0 + 64, :],
                                ss_sb[o0:o0 + 32, qs])
                            nc.vector.tensor_mul(
                                t2[o0 + 32:o0 + 64, :], pj[o0:o0 + 32, :],
                                ss_sb[o0 + 32:o0 + 64, qs])
                        nc.gpsimd.tensor_add(qk_sb[:, blk, qs], t1, t2)

            def emit_v(tt):
                vp = ps_y.tile([P, 256], F32, tag="y")
                for kc in range(NCC):
                    nc.tensor.matmul(
                        vp,
                        lhsT=xt_sb[:, kc, tt * P:(tt + 1) * P],
                        rhs=wv_sb[:, kc, :],
                        start=(kc == 0),
                        stop=(kc == NCC - 1),
                    )
                vp4 = vp.rearrange("p (g h d) -> p g h d", g=2, h=2)
                nc.vector.tensor_copy(v_sb4[:, tt, :, :, 1, :], vp4)

            def emit_attn(grp, qt):
                q_t = qk_sb[:, grp, :]
                k_t = qk_sb[:, 2 + grp, :]
                qs = slice(qt * QT, (qt + 1) * QT)
                KC = 4 * (qt + 1)  # causal k chunks
                yA = ps_y.tile([P, QT], F32, tag="y")
                yB = ps_y.tile([P, QT], F32, tag="y")
                for kc in range(KC):
                    ks = slice(kc * P, (kc + 1) * P)
                    ps = ps_sc.tile([P, 1024], F32, tag="sc")
                    nc.tensor.matmul(ps[:, 0:QT], lhsT=k_t[0:64, ks],
                                     rhs=q_t[0:64, qs],
                                     start=True, stop=True)
                    nc.tensor.matmul(ps[:, QT:1024], lhsT=k_t[64:128, ks],
                                     rhs=q_t[64:128, qs],
                                     start=True, stop=True)
                    e = epool.tile([P, 1024], BF, tag="e")
                    delta = kc * P - qt * QT
                    if delta <= 0:
                        nc.scalar.activation(e, ps, AF.Exp,
                                             scale=sce_sb[:, 0:1])
                    else:
                        # diagonal tile: columns below delta are fully
                        # masked; exp only the live range, zero the rest
                        e2 = e.rearrange("p (j f) -> p j f", j=2)
                        p2 = ps.rearrange("p (j f) -> p j f", j=2)
                        nc.gpsimd.memset(e2[:, :, 0:delta], 0.0)
                        nc.scalar.activation(e2[:, :, delta:QT],
                                             p2[:, :, delta:QT],
                                             AF.Exp, scale=sce_sb[:, 0:1])
                    if delta >= 0:
                        # keep where f - p - delta >= 0
                        e2 = e.rearrange("p (j f) -> p j f", j=2)
                        nc.gpsimd.affine_select(
                            e2, e2,
                            pattern=[[0, 2], [1, QT]],
                            compare_op=mybir.AluOpType.is_ge,
                            fill=0.0,
                            base=-delta,
                            channel_multiplier=-1,
                        )
                    nc.tensor.matmul(yA, lhsT=v_sb[:, kc, grp, 0:128],
                                     rhs=e[:, 0:QT],
                                     start=(kc == 0), stop=(kc == KC - 1))
                    nc.tensor.matmul(yB, lhsT=v_sb[:, kc, grp, 128:256],
                                     rhs=e[:, QT:1024],
                                     start=(kc == 0), stop=(kc == KC - 1))
                # both heads: denom rows 0:64, y rows 64:128
                rcA = tmps.tile([P, QT], F32, tag="rc")
                nc.vector.reciprocal_approx_fast(rcA[0:64, :], yA[0:64, :])
                nc.vector.tensor_mul(y_sb[0:64, grp, qs], yA[64:128, :],
                                     rcA[0:64, :])
                rcB = tmps.tile([P, QT], F32, tag="rc")
                nc.vector.reciprocal_approx_fast(rcB[0:64, :], yB[0:64, :])
                nc.vector.tensor_copy(rcB[64:128, :], rcB[0:64, :])
                nc.vector.tensor_mul(y_sb[64:128, grp, qs], yB[64:128, :],
                                     rcB[64:128, :])

            def emit_proj(qt):
                qs = slice(qt * QT, (qt + 1) * QT)
                for mt in range(8):
                    ms = slice(mt * P, (mt + 1) * P)
                    pp = ps_y.tile([P, QT], F32, tag="y")
                    for ch in range(2):
                        nc.tensor.matmul(pp, lhsT=wp_sb[:, ch, ms],
                                         rhs=y_sb[:, ch, qs],
                                         start=(ch == 0), stop=(ch == 1))
                    ot = opool.tile([P, QT], F32, tag="ot")
                    if mt % 2 == 0:
                        nc.scalar.activation(ot, pp, AF.Copy,
                                             scale=sco_sb[:, 0:1])
                    else:
                        nc.vector.tensor_scalar_mul(ot, pp, sco_sb[:, 0:1])
                    eng = (nc.sync, nc.gpsimd)[mt % 2]
                    eng.dma_start(out=outT[ms, qs], in_=ot)

            # qt order (1,2,3,0): prefix deps stay legal while the kernel
            # tail is the lightest attention tile + its projection
            emit_qkv(0)
            emit_qkv(1)
            for tt in range(8):
                emit_v(tt)
            emit_attn(0, 1)
            emit_qkv(2)
            emit_attn(1, 1)
            for tt in range(8, 12):
                emit_v(tt)
            emit_proj(1)
            emit_attn(0, 2)
            emit_qkv(3)
            emit_attn(1, 2)
            for tt in range(12, 16):
                emit_v(tt)
            emit_proj(2)
            emit_attn(0, 3)
            emit_attn(1, 3)
            emit_proj(3)
            emit_attn(0, 0)
            emit_attn(1, 0)
            emit_proj(0)

    nc.finalize()
    return nc


def _prep_inputs(x, cos, sin, w_qkv, w_proj):
    sq, am_q = _ternarize_host(w_qkv)
    sp, am_p = _ternarize_host(w_proj)

    cos_t = np.ascontiguousarray(cos[0, 0].T).astype(np.float32)  # [D, T]
    sin_t = np.ascontiguousarray(sin[0, 0].T).astype(np.float32)
    sgn = np.where(np.arange(D)[:, None] < D // 2, np.float32(-1.0),
                   np.float32(1.0))
    ss_t = sin_t * sgn
    cos2 = np.concatenate([cos_t, cos_t], axis=0)          # [128, T]
    ss2 = np.concatenate([ss_t, ss_t], axis=0)
    sc_exp = np.full((P, 1), am_q * am_q / np.sqrt(np.float32(D)),
                     np.float32)
    sc_out = np.full((P, 1), am_q * am_p, np.float32)

    perm = (np.arange(D) + D // 2) % D
    in_maps = []
    for core in range(N_CORES):
        b, g = divmod(core, HEADS_PER_CORE)
        heads = [4 * g + h for h in range(4)]
        q_rows = np.concatenate([np.arange(h * D, (h + 1) * D) for h in heads])
        k_rows = C + q_rows
        v_rows = 2 * C + q_rows
        wqk_block = np.concatenate([sq[q_rows], sq[k_rows]], axis=0)
        wqk_t = np.ascontiguousarray(wqk_block.T).astype(BF16)   # [C, 1024]
        wv_t = np.ascontiguousarray(sq[v_rows].T).astype(BF16)   # [C, 256]
        wp_t = np.ascontiguousarray(sp[:, q_rows].T).astype(BF16)  # [256, C]
        xt = np.ascontiguousarray(x[b].T).astype(BF16)           # [C, T]
        in_maps.append({
            "xt": xt, "wqk": wqk_t, "wv": wv_t, "wp": wp_t,
            "cos2": cos2, "ss2": ss2, "sc_exp": sc_exp, "sc_out": sc_out,
        })
    return in_maps


def kernel(x, cos, sin, w_qkv, w_proj):
    x = np.asarray(x, dtype=np.float32)
    cos = np.asarray(cos, dtype=np.float32)
    sin = np.asarray(sin, dtype=np.float32)
    w_qkv = np.asarray(w_qkv, dtype=np.float32)
    w_proj = np.asarray(w_proj, dtype=np.float32)

    _install_ntff_hook()
    from concourse.bass_utils import run_bass_kernel_spmd

    if "nc" not in _CACHE:
        _CACHE["nc"] = _build_program()
    nc = _CACHE["nc"]

    in_maps = _prep_inputs(x, cos, sin, w_qkv, w_proj)
    trace = bool(os.environ.get("KERNEL_TRACE"))
    res = run_bass_kernel_spmd(nc, in_maps, core_ids=list(range(N_CORES)),
                               trace=trace)
    _CACHE["exec_time_ns"] = res.exec_time_ns

    out = np.zeros((B, T, C), dtype=np.float32)
    for core in range(N_CORES):
        b = core // HEADS_PER_CORE
        out[b] += res.results[core]["outT"].T
    return out
